# revision 29
# baseline (speedup 1.0000x reference)
import sys

import numpy as np

sys.path.insert(0, "/opt/trn_rl_repo")

from concourse import bacc, bass, tile  # noqa: E402,F401
from concourse import mybir  # noqa: E402
from concourse.bass import broadcast_tensor_aps  # noqa: E402
from concourse.bass_utils import run_bass_kernel_spmd  # noqa: E402
from concourse.masks import make_identity  # noqa: E402

N_CORES = 8
S = 8  # samples per core
C = 3
T = 9
H = W = 256
RC = 4  # rows per chunk (one SBUF partition holds one chunk)
NCH = H // RC  # 64 chunks per sample
RP = RC + 2  # row slots incl top/bottom halo
CB = RP * W + 2  # channel block: 6 row slots + phantom head/tail columns
F32 = mybir.dt.float32
F16 = mybir.dt.float16
NPROD = 4  # product ring depth
# row-1 taps first: they need neither halo rows nor anything beyond the
# main x DMA + their sigma plane, so compute starts as early as possible
TAP_ORDER = [4, 3, 5, 1, 7, 0, 2, 6, 8]


def build_nc():
    nc = bacc.Bacc()
    x_ext = nc.declare_dram_parameter("x", [S, C, H, W], F16, isOutput=False)
    sg_ext = nc.declare_dram_parameter("sigma", [S, T, H, W], F16, isOutput=False)
    out_ext = nc.declare_dram_parameter("out", [S, C, H, W], F16, isOutput=True)

    with tile.TileContext(nc) as tc:
        with (
            tc.tile_pool(name="const", bufs=1) as cpool,
            tc.tile_pool(name="p", bufs=2) as pool,
            tc.tile_pool(name="ps", bufs=1, space="PSUM") as psum,
        ):
            ident = cpool.tile([128, 128], F16)
            make_identity(nc, ident[:])

            # stripe N's normalize+store runs inside stripe N+1's mul
            # stream (software pipeline): by then the ActE downcasts of
            # stripe N's PSUM accumulators have long finished
            prev_tail = None

            for stripe in range(S // 2):
                # x rows stored contiguously (no column pads): DMA packets are
                # 2KB instead of 512B, ~12x better per-queue DMA throughput.
                # Host-side sigma edge-shuffling (see _run) makes every tap's
                # mul full-width: the out-of-image x columns are multiplied
                # by zeroed sigma edge columns, and their true reflect
                # contribution rides on the opposite tap of the same row,
                # which reads exactly the right x element at the edge.
                xtp = pool.tile([128, C, CB], F16)
                st = pool.tile([128, T, RC, W], F16)
                prods = [
                    pool.tile([128, C, RC, W], F16, name=f"prod{j}")
                    for j in range(NPROD)
                ]
                acc16 = pool.tile([128, C, RC, W], F16)
                inv32 = pool.tile([128, RC, W], F32)
                inv = pool.tile([128, RC, W], F16)
                ot = pool.tile([128, C, RC, W], F16)
                # 3 acc channels + den: 4 tiles x 2 PSUM banks = all 8 banks
                accs = [
                    psum.tile([128, RC, W], F32, name=f"acc{c}") for c in range(C)
                ]
                den_ps = psum.tile([128, RC, W], F32)

                # phantom columns must read as finite (0 x garbage = 0, but
                # 0 x NaN is NaN): zero them; nothing ever overwrites them
                nc.gpsimd.memset(xtp[:, :, 0:1], 0.0)
                nc.gpsimd.memset(xtp[:, :, CB - 1 : CB], 0.0)

                for k in range(2):
                    s = 2 * stripe + k
                    pb = 64 * k
                    # disjoint partition halves -> run the two samples' DMAs
                    # on separate engine queues
                    eng = nc.sync if k == 0 else nc.scalar
                    xr = x_ext[s].rearrange("c (n r) w -> n c r w", r=RC)
                    xr2 = x_ext[s].rearrange("c (n r) w -> n c (r w)", r=RC)
                    sr = sg_ext[s].rearrange("t (n r) w -> n t (r w)", r=RC)
                    # center-tap sigma first, then main x rows per channel.
                    # Keep the loads as MANY SMALL transfers in consumption
                    # order: the DMA pool fair-shares bandwidth across all
                    # in-flight transfers, so small early-issued DMAs complete
                    # earliest -- issue order acts as a priority scheme.
                    # (Batching sigma into one 1.18MB DMA per half measured
                    # +14us: the whole transfer then lands fair-share-late.)
                    eng.dma_start(st[pb : pb + 64, 4], sr[:, 4])
                    for c in range(C):
                        eng.dma_start(
                            xtp[pb : pb + 64, c, 1 + W : 1 + 5 * W], xr2[:, c]
                        )
                    # halo rows go via the otherwise idle gpsimd sequencer so
                    # they start streaming early;
                    # top halo: chunks 1..63 read prev chunk row 3
                    nc.gpsimd.dma_start(
                        xtp[pb + 1 : pb + 64, :, 1 : 1 + W], xr[0:63, :, 3, :]
                    )
                    # chunk 0 top halo: reflect row 1
                    nc.gpsimd.dma_start(
                        xtp[pb : pb + 1, :, 1 : 1 + W], xr[0:1, :, 1, :]
                    )
                    # bottom halo row: chunks 0..62 read next chunk row 0
                    nc.gpsimd.dma_start(
                        xtp[pb : pb + 63, :, 1 + 5 * W : 1 + 6 * W],
                        xr[1:64, :, 0, :],
                    )
                    # chunk 63 bottom halo: reflect row 254 (= chunk 63 row 2)
                    nc.gpsimd.dma_start(
                        xtp[pb + 63 : pb + 64, :, 1 + 5 * W : 1 + 6 * W],
                        xr[63:64, :, 2, :],
                    )
                    # remaining sigma planes one-by-one in consumption order
                    for t in TAP_ORDER[1:]:
                        eng.dma_start(st[pb : pb + 64, t], sr[:, t])

                # DVE computes only the 9 per-tap products; the otherwise-idle
                # PE accumulates them (and the sigma sum) into PSUM via
                # identity-stationary matmuls: start=first tap resets, the
                # rest accumulate. Matmul moving free dim is capped at 512
                # (= one PSUM bank), so each [128,C,RC,W] plane is 6 slices
                # and each den plane is 2.
                with nc.allow_low_precision(reason="fp16 kernel"):
                    for j, t in enumerate(TAP_ORDER):
                        if j == 2 and prev_tail is not None:
                            prev_tail()
                            prev_tail = None
                        di, dj = t // 3, t % 3
                        prod = prods[j % NPROD]
                        off = di * W + dj
                        xs = xtp[:, :, off : off + RC * W].rearrange(
                            "p c (r w) -> p c r w", w=W
                        )
                        if j == 0 and stripe == 0:
                            # cold start only: split per channel so compute
                            # starts as soon as each channel's DMA lands; on
                            # prefetched stripes the batched op is cheaper
                            # (one instruction overhead instead of three)
                            for c in range(C):
                                nc.vector.tensor_mul(
                                    prod[:, c], xs[:, c], st[:, t, :, :]
                                )
                        else:
                            a, b = broadcast_tensor_aps(xs, st[:, t : t + 1])
                            nc.vector.tensor_mul(prod[:], a, b)

                        first, last = j == 0, j == T - 1
                        # den slices first: they only need the sigma DMA, so
                        # PE can run them even while DVE waits on x
                        for r in range(0, RC, 2):
                            nc.tensor.matmul(
                                den_ps[:, r : r + 2, :],
                                ident[:],
                                st[:, t, r : r + 2, :],
                                start=first,
                                stop=last,
                            )
                        for c in range(C):
                            for r in range(0, RC, 2):
                                nc.tensor.matmul(
                                    accs[c][:, r : r + 2, :],
                                    ident[:],
                                    prod[:, c, r : r + 2, :],
                                    start=first,
                                    stop=last,
                                )

                    # ~5x faster than reciprocal(); ~18 correct bits and
                    # den in [0.5, 9] so no edge cases. eps=1e-9 is far
                    # below fp16 noise -> dropped.
                    nc.vector.reciprocal_approx_fast(inv32[:], den_ps[:])

                    # ActE downcasts the PSUM accumulators to fp16 (frees the
                    # PSUM banks for the next stripe and lets the DVE
                    # normalize run in the 2x perf mode); acc0 first so the
                    # next stripe's PE can start on bank 0 soonest
                    nc.scalar.copy(acc16[:, 0], accs[0][:])
                    nc.scalar.copy(inv[:], inv32[:])
                    nc.scalar.copy(acc16[:, 1], accs[1][:])
                    nc.scalar.copy(acc16[:, 2], accs[2][:])

                    def make_tail(stripe, acc16, inv, ot):
                        def tail():
                            for c in range(C):
                                nc.vector.tensor_mul(ot[:, c], acc16[:, c], inv[:])
                                for k in range(2):
                                    s = 2 * stripe + k
                                    pb = 64 * k
                                    eng = nc.sync if k == 0 else nc.scalar
                                    orr = out_ext[s].rearrange(
                                        "c (n r) w -> n c r w", r=RC
                                    )
                                    eng.dma_start(orr[:, c], ot[pb : pb + 64, c])

                        return tail

                    prev_tail = make_tail(stripe, acc16, inv, ot)

            with nc.allow_low_precision(reason="fp16 kernel"):
                prev_tail()

    nc.finalize()
    return nc


_nc_cache = None


def _get_nc():
    global _nc_cache
    if _nc_cache is None:
        _nc_cache = build_nc()
    return _nc_cache


def _shuffle_sigma_edges(sigma):
    """Move each row's reflect-column mass between the left/right taps.

    At w=0 the left tap (3di) needs x[:,1] (reflection), which is exactly
    what the right tap (3di+2) reads there; and vice versa at w=W-1. So
    adding the left tap's edge weight onto the right tap (and zeroing the
    left) makes plain shifted full-width products exact. The per-pixel tap
    sum (the normalizer) is unchanged.
    """
    sigma = sigma.copy()
    for di in range(3):
        left, right = sigma[:, 3 * di], sigma[:, 3 * di + 2]
        l0 = left[..., 0].copy()
        r1 = right[..., W - 1].copy()
        right[..., 0] += l0
        left[..., 0] = 0
        left[..., W - 1] += r1
        right[..., W - 1] = 0
    return sigma


def _run(x, sigma, trace=False):
    x = np.ascontiguousarray(x).astype(np.float16)
    sigma = _shuffle_sigma_edges(np.ascontiguousarray(sigma).astype(np.float16))
    nc = _get_nc()
    in_maps = [
        {"x": x[S * i : S * (i + 1)], "sigma": sigma[S * i : S * (i + 1)]}
        for i in range(N_CORES)
    ]
    res = run_bass_kernel_spmd(nc, in_maps, list(range(N_CORES)), trace=trace)
    out = np.concatenate([res.results[i]["out"] for i in range(N_CORES)], axis=0)
    return out.astype(np.float32), res


def kernel(x, sigma):
    out, _ = _run(x, sigma)
    return out


# revision 30
# speedup vs baseline: 1.0317x; 1.0317x over previous
import sys

import numpy as np

sys.path.insert(0, "/opt/trn_rl_repo")

from concourse import bacc, bass, tile  # noqa: E402,F401
from concourse import mybir  # noqa: E402
from concourse.bass import broadcast_tensor_aps  # noqa: E402
from concourse.bass_utils import run_bass_kernel_spmd  # noqa: E402
from concourse.masks import make_identity  # noqa: E402

N_CORES = 8
S = 8  # samples per core
C = 3
T = 9
H = W = 256
RC = 4  # rows per chunk (one SBUF partition holds one chunk)
NCH = H // RC  # 64 chunks per sample
RP = RC + 2  # row slots incl top/bottom halo
CB = RP * W + 2  # channel block: 6 row slots + phantom head/tail columns
F32 = mybir.dt.float32
F16 = mybir.dt.float16
NPROD = 4  # product ring depth
# row-1 taps first: they need neither halo rows nor anything beyond the
# main x DMA + their sigma plane, so compute starts as early as possible
TAP_ORDER = [4, 3, 5, 1, 7, 0, 2, 6, 8]


def build_nc():
    nc = bacc.Bacc()
    x_ext = nc.declare_dram_parameter("x", [S, C, H, W], F16, isOutput=False)
    sg_ext = nc.declare_dram_parameter("sigma", [S, T, H, W], F16, isOutput=False)
    out_ext = nc.declare_dram_parameter("out", [S, C, H, W], F16, isOutput=True)

    with tile.TileContext(nc) as tc:
        with (
            tc.tile_pool(name="const", bufs=1) as cpool,
            tc.tile_pool(name="p", bufs=2) as pool,
            tc.tile_pool(name="ps", bufs=1, space="PSUM") as psum,
        ):
            ident = cpool.tile([128, 128], F16)
            make_identity(nc, ident[:])

            # stripe N's normalize+store runs inside stripe N+1's mul
            # stream (software pipeline): by then the ActE downcasts of
            # stripe N's PSUM accumulators have long finished
            prev_tail = None

            for stripe in range(S // 2):
                # x rows stored contiguously (no column pads): DMA packets are
                # 2KB instead of 512B, ~12x better per-queue DMA throughput.
                # Host-side sigma edge-shuffling (see _run) makes every tap's
                # mul full-width: the out-of-image x columns are multiplied
                # by zeroed sigma edge columns, and their true reflect
                # contribution rides on the opposite tap of the same row,
                # which reads exactly the right x element at the edge.
                xtp = pool.tile([128, C, CB], F16)
                st = pool.tile([128, T, RC, W], F16)
                prods = [
                    pool.tile([128, C, RC, W], F16, name=f"prod{j}")
                    for j in range(NPROD)
                ]
                acc16 = pool.tile([128, C, RC, W], F16)
                inv32 = pool.tile([128, RC, W], F32)
                inv = pool.tile([128, RC, W], F16)
                ot = pool.tile([128, C, RC, W], F16)
                # 3 acc channels + den: 4 tiles x 2 PSUM banks = all 8 banks
                accs = [
                    psum.tile([128, RC, W], F32, name=f"acc{c}") for c in range(C)
                ]
                den_ps = psum.tile([128, RC, W], F32)

                # phantom columns must read as finite (0 x garbage = 0, but
                # 0 x NaN is NaN): zero them; nothing ever overwrites them
                nc.gpsimd.memset(xtp[:, :, 0:1], 0.0)
                nc.gpsimd.memset(xtp[:, :, CB - 1 : CB], 0.0)

                for k in range(2):
                    s = 2 * stripe + k
                    pb = 64 * k
                    # disjoint partition halves -> run the two samples' DMAs
                    # on separate engine queues
                    eng = nc.sync if k == 0 else nc.scalar
                    xr = x_ext[s].rearrange("c (n r) w -> n c r w", r=RC)
                    xr2 = x_ext[s].rearrange("c (n r) w -> n c (r w)", r=RC)
                    sr = sg_ext[s].rearrange("t (n r) w -> n t (r w)", r=RC)
                    # center-tap sigma first, then main x rows per channel.
                    # Keep the loads as MANY SMALL transfers in consumption
                    # order: the DMA pool fair-shares bandwidth across all
                    # in-flight transfers, so small early-issued DMAs complete
                    # earliest -- issue order acts as a priority scheme.
                    # (Batching sigma into one 1.18MB DMA per half measured
                    # +14us: the whole transfer then lands fair-share-late.)
                    eng.dma_start(st[pb : pb + 64, 4], sr[:, 4])
                    for c in range(C):
                        eng.dma_start(
                            xtp[pb : pb + 64, c, 1 + W : 1 + 5 * W], xr2[:, c]
                        )
                    # halo rows go via the otherwise idle gpsimd sequencer so
                    # they start streaming early;
                    # top halo: chunks 1..63 read prev chunk row 3
                    nc.gpsimd.dma_start(
                        xtp[pb + 1 : pb + 64, :, 1 : 1 + W], xr[0:63, :, 3, :]
                    )
                    # chunk 0 top halo: reflect row 1
                    nc.gpsimd.dma_start(
                        xtp[pb : pb + 1, :, 1 : 1 + W], xr[0:1, :, 1, :]
                    )
                    # bottom halo row: chunks 0..62 read next chunk row 0
                    nc.gpsimd.dma_start(
                        xtp[pb : pb + 63, :, 1 + 5 * W : 1 + 6 * W],
                        xr[1:64, :, 0, :],
                    )
                    # chunk 63 bottom halo: reflect row 254 (= chunk 63 row 2)
                    nc.gpsimd.dma_start(
                        xtp[pb + 63 : pb + 64, :, 1 + 5 * W : 1 + 6 * W],
                        xr[63:64, :, 2, :],
                    )
                    # remaining sigma planes one-by-one in consumption order
                    for t in TAP_ORDER[1:]:
                        eng.dma_start(st[pb : pb + 64, t], sr[:, t])

                # DVE computes only the 9 per-tap products; the otherwise-idle
                # PE accumulates them (and the sigma sum) into PSUM via
                # identity-stationary matmuls: start=first tap resets, the
                # rest accumulate. Matmul moving free dim is capped at 512
                # (= one PSUM bank), so each [128,C,RC,W] plane is 6 slices
                # and each den plane is 2.
                with nc.allow_low_precision(reason="fp16 kernel"):
                    for j, t in enumerate(TAP_ORDER):
                        if j == 2 and prev_tail is not None:
                            prev_tail()
                            prev_tail = None
                        di, dj = t // 3, t % 3
                        prod = prods[j % NPROD]
                        off = di * W + dj
                        xs = xtp[:, :, off : off + RC * W].rearrange(
                            "p c (r w) -> p c r w", w=W
                        )
                        if j == 0:
                            # split per channel: compute starts as soon as
                            # each channel's DMA lands, not all three
                            for c in range(C):
                                nc.vector.tensor_mul(
                                    prod[:, c], xs[:, c], st[:, t, :, :]
                                )
                        else:
                            a, b = broadcast_tensor_aps(xs, st[:, t : t + 1])
                            nc.vector.tensor_mul(prod[:], a, b)

                        first, last = j == 0, j == T - 1
                        # den slices first: they only need the sigma DMA, so
                        # PE can run them even while DVE waits on x
                        for r in range(0, RC, 2):
                            nc.tensor.matmul(
                                den_ps[:, r : r + 2, :],
                                ident[:],
                                st[:, t, r : r + 2, :],
                                start=first,
                                stop=last,
                            )
                        for c in range(C):
                            for r in range(0, RC, 2):
                                nc.tensor.matmul(
                                    accs[c][:, r : r + 2, :],
                                    ident[:],
                                    prod[:, c, r : r + 2, :],
                                    start=first,
                                    stop=last,
                                )

                    # ~5x faster than reciprocal(); ~18 correct bits and
                    # den in [0.5, 9] so no edge cases. eps=1e-9 is far
                    # below fp16 noise -> dropped.
                    nc.vector.reciprocal_approx_fast(inv32[:], den_ps[:])

                    # ActE downcasts the PSUM accumulators to fp16 (frees the
                    # PSUM banks for the next stripe and lets the DVE
                    # normalize run in the 2x perf mode); acc0 first so the
                    # next stripe's PE can start on bank 0 soonest
                    nc.scalar.copy(acc16[:, 0], accs[0][:])
                    nc.scalar.copy(inv[:], inv32[:])
                    nc.scalar.copy(acc16[:, 1], accs[1][:])
                    nc.scalar.copy(acc16[:, 2], accs[2][:])

                    def make_tail(stripe, acc16, inv, ot):
                        def tail():
                            for c in range(C):
                                nc.vector.tensor_mul(ot[:, c], acc16[:, c], inv[:])
                                for k in range(2):
                                    s = 2 * stripe + k
                                    pb = 64 * k
                                    eng = nc.sync if k == 0 else nc.scalar
                                    orr = out_ext[s].rearrange(
                                        "c (n r) w -> n c r w", r=RC
                                    )
                                    eng.dma_start(orr[:, c], ot[pb : pb + 64, c])

                        return tail

                    prev_tail = make_tail(stripe, acc16, inv, ot)

            with nc.allow_low_precision(reason="fp16 kernel"):
                prev_tail()

    nc.finalize()
    return nc


_nc_cache = None


def _get_nc():
    global _nc_cache
    if _nc_cache is None:
        _nc_cache = build_nc()
    return _nc_cache


def _shuffle_sigma_edges(sigma):
    """Move each row's reflect-column mass between the left/right taps.

    At w=0 the left tap (3di) needs x[:,1] (reflection), which is exactly
    what the right tap (3di+2) reads there; and vice versa at w=W-1. So
    adding the left tap's edge weight onto the right tap (and zeroing the
    left) makes plain shifted full-width products exact. The per-pixel tap
    sum (the normalizer) is unchanged.
    """
    sigma = sigma.copy()
    for di in range(3):
        left, right = sigma[:, 3 * di], sigma[:, 3 * di + 2]
        l0 = left[..., 0].copy()
        r1 = right[..., W - 1].copy()
        right[..., 0] += l0
        left[..., 0] = 0
        left[..., W - 1] += r1
        right[..., W - 1] = 0
    return sigma


def _run(x, sigma, trace=False):
    x = np.ascontiguousarray(x).astype(np.float16)
    sigma = _shuffle_sigma_edges(np.ascontiguousarray(sigma).astype(np.float16))
    nc = _get_nc()
    in_maps = [
        {"x": x[S * i : S * (i + 1)], "sigma": sigma[S * i : S * (i + 1)]}
        for i in range(N_CORES)
    ]
    res = run_bass_kernel_spmd(nc, in_maps, list(range(N_CORES)), trace=trace)
    out = np.concatenate([res.results[i]["out"] for i in range(N_CORES)], axis=0)
    return out.astype(np.float32), res


def kernel(x, sigma):
    out, _ = _run(x, sigma)
    return out
